# revision 8
# baseline (speedup 1.0000x reference)
"""Bahdanau attention kernel for Trainium2 (8 NeuronCores, batch-parallel).

reference computation (B=64, L=4096, D=512, U=256):
    f_proj = features @ W1 + b1                    # [B, L, U]
    h_proj = hidden @ W2 + b2                      # [B, U]
    score  = tanh(f_proj + h_proj[:, None]) @ V + Vb   # [B, L, 1]
    attn   = softmax(score, axis=1)
    ctx    = sum(attn * features, axis=1)          # [B, D]
    return (ctx, attn)

Strategy: shard B across 8 cores (8 batches/core). Host pre-marshals two bf16
copies of features — natural [b, l, d] (for the context contraction over l)
and transposed [b, d, l] (for the f_proj contraction over d) — since the PE
contracts over the partition dim and an on-chip full transpose is slower than
streaming a second copy from HBM. Softmax is computed without max-subtraction
(scores are bounded by sum|V| ~ 8, exp stays finite in fp32), which makes the
whole thing a single fused streaming pass: raw exp(score) accumulates the
context matmul in PSUM and both outputs are normalized by 1/sum at the end.
Vb shifts every score in a batch equally, so softmax makes it a no-op and it
is dropped.

Per-core layouts (P=128 partitions):
  scores are produced directly in column layout e[p, j] = exp(score[j*128+p])
  via matmuls lhsT=tanh_tile[u, m-chunk], rhs=V[u, 1] accumulating over the
  two u-chunks; that column is exactly the lhsT the context matmul needs.
  The attention-weights output row is recovered with one 128x32 PE transpose
  per batch.
"""

import sys
from contextlib import ExitStack

if "/opt/trn_rl_repo" not in sys.path:
    sys.path.insert(0, "/opt/trn_rl_repo")

import numpy as np
import ml_dtypes

BF16 = ml_dtypes.bfloat16

B_FULL, L, D, U = 64, 4096, 512, 256
NCORES = 8
BSH = B_FULL // NCORES  # batches per core
P = 128

_cache = {}


def build_program(bsh=BSH, l=L, d=D, u=U, dma_split=2):
    import concourse.bacc as bacc
    import concourse.tile as tile
    from concourse import mybir
    from concourse.masks import make_identity

    dt = mybir.dt
    DC = d // P            # d-chunks (4)
    UC = u // P            # u-chunks (2)
    NCH = l // P           # l-chunks per batch (32)
    BLK = 512              # m-block for the f_proj matmuls
    NBLK = l // BLK        # blocks per batch (8)
    CPB = BLK // P         # l-chunks per block (4)
    MH = l // dma_split    # m per DMA (2048)
    CPH = MH // P          # l-chunks per DMA half (16)
    BPH = MH // BLK        # blocks per DMA half (4)

    nc = bacc.Bacc("TRN2", target_bir_lowering=False, debug=False,
                   num_devices=NCORES)

    ft = nc.dram_tensor("ft", [bsh, d, l], dt.bfloat16, kind="ExternalInput")
    fn = nc.dram_tensor("fn", [bsh, l, d], dt.bfloat16, kind="ExternalInput")
    ht = nc.dram_tensor("ht", [d, bsh], dt.float32, kind="ExternalInput")
    w1 = nc.dram_tensor("w1", [d, u], dt.bfloat16, kind="ExternalInput")
    w2 = nc.dram_tensor("w2", [d, u], dt.float32, kind="ExternalInput")
    bs = nc.dram_tensor("bs", [u], dt.float32, kind="ExternalInput")
    vk = nc.dram_tensor("vk", [u], dt.bfloat16, kind="ExternalInput")
    ctx_out = nc.dram_tensor("ctx", [bsh, d], dt.float32, kind="ExternalOutput")
    attn_out = nc.dram_tensor("attn", [bsh, l], dt.float32, kind="ExternalOutput")

    ft_src = ft.ap().rearrange("b (dc p) l -> b p dc l", p=P)
    fn_src = fn.ap().rearrange("b (s p) d -> b p s d", p=P)
    attn_dst = attn_out.ap().rearrange("b (j p) -> b j p", p=P)

    with tile.TileContext(nc) as tc, ExitStack() as es:
        const = es.enter_context(tc.tile_pool(name="const", bufs=1))
        ftp = es.enter_context(tc.tile_pool(name="ftp", bufs=3))
        fnp = es.enter_context(tc.tile_pool(name="fnp", bufs=dma_split + 2))
        ttp = es.enter_context(tc.tile_pool(name="ttp", bufs=3))
        small = es.enter_context(tc.tile_pool(name="small", bufs=4))
        rows = es.enter_context(tc.tile_pool(name="rows", bufs=2))
        fproj_ps = es.enter_context(tc.tile_pool(name="fproj_ps", bufs=2, space="PSUM"))
        scores_psp = es.enter_context(tc.tile_pool(name="scores_ps", bufs=1, space="PSUM"))
        srow_psp = es.enter_context(tc.tile_pool(name="srow_ps", bufs=2, space="PSUM"))
        ctx_psp = es.enter_context(tc.tile_pool(name="ctx_ps", bufs=1, space="PSUM"))
        epi_psp = es.enter_context(tc.tile_pool(name="epi_ps", bufs=1, space="PSUM"))

        # ---- constants ----
        w1_sb = const.tile([P, DC, u], dt.bfloat16, name="w1_sb")
        nc.sync.dma_start(out=w1_sb, in_=w1.ap().rearrange("(dc p) u -> p dc u", p=P))
        w2_sb = const.tile([P, DC, u], dt.float32, name="w2_sb")
        nc.sync.dma_start(out=w2_sb, in_=w2.ap().rearrange("(dc p) u -> p dc u", p=P))
        ht_sb = const.tile([P, DC, bsh], dt.float32, name="ht_sb")
        nc.sync.dma_start(out=ht_sb, in_=ht.ap().rearrange("(dc p) b -> p dc b", p=P))
        v_sb = const.tile([P, UC], dt.bfloat16, name="v_sb")
        nc.sync.dma_start(out=v_sb, in_=vk.ap().rearrange("(uc p) -> p uc", p=P))
        bs_sb = const.tile([P, UC], dt.float32, name="bs_sb")
        nc.sync.dma_start(out=bs_sb, in_=bs.ap().rearrange("(uc p) -> p uc", p=P))
        ones_sb = const.tile([P, 32], dt.float32, name="ones_sb")
        nc.vector.memset(ones_sb, 1.0)
        ident_sb = const.tile([P, P], dt.float32, name="ident_sb")
        make_identity(nc, ident_sb)
        qt_sb = const.tile([P, UC, bsh], dt.float32, name="qt_sb")

        # ---- h_proj (tiny): qT[u, b] = (W2.T @ hT)[u, b] + b1[u] + b2[u] ----
        for uc in range(UC):
            q_ps = epi_psp.tile([P, bsh], dt.float32, tag="et", name=f"q_ps{uc}")
            for dc in range(DC):
                nc.tensor.matmul(q_ps, lhsT=w2_sb[:, dc, uc * P:(uc + 1) * P],
                                 rhs=ht_sb[:, dc, :],
                                 start=(dc == 0), stop=(dc == DC - 1))
            nc.vector.tensor_scalar_add(qt_sb[:, uc, :], q_ps,
                                        bs_sb[:, uc:uc + 1])

        # ---- main loop over this core's batches ----
        for b in range(bsh):
            ft_tiles = []
            fn_tiles = []
            for h in range(dma_split):
                ftt = ftp.tile([P, DC, MH], dt.bfloat16, tag="ft", name=f"ft{b}_{h}")
                nc.sync.dma_start(out=ftt, in_=ft_src[b, :, :, h * MH:(h + 1) * MH])
                ft_tiles.append(ftt)
                fnt = fnp.tile([P, CPH, d], dt.bfloat16, tag="fn", name=f"fn{b}_{h}")
                nc.sync.dma_start(out=fnt, in_=fn_src[b, :, h * CPH:(h + 1) * CPH, :])
                fn_tiles.append(fnt)

            scores_ps = scores_psp.tile([P, NCH], dt.float32, tag="scores",
                                        name=f"scores{b}")
            slab = rows.tile([NBLK, BLK], dt.float32, tag="slab",
                             name=f"slab{b}")
            rowfull = rows.tile([1, l], dt.float32, tag="rowfull",
                                name=f"rowfull{b}")
            for blk in range(NBLK):
                h, moff = blk // BPH, (blk % BPH) * BLK
                ftt = ft_tiles[h]
                fp_ps = []
                for uc in range(UC):
                    fp = fproj_ps.tile([P, BLK], dt.float32, tag="fproj",
                                       name=f"fp{b}_{blk}_{uc}")
                    for dc in range(DC):
                        nc.tensor.matmul(fp, lhsT=w1_sb[:, dc, uc * P:(uc + 1) * P],
                                         rhs=ftt[:, dc, moff:moff + BLK],
                                         start=(dc == 0), stop=(dc == DC - 1))
                    fp_ps.append(fp)
                tt = ttp.tile([P, UC, BLK], dt.bfloat16, tag="tt",
                              name=f"tt{b}_{blk}")
                for uc in range(UC):
                    nc.scalar.activation(out=tt[:, uc, :], in_=fp_ps[uc],
                                         func=mybir.ActivationFunctionType.Tanh,
                                         bias=qt_sb[:, uc, b:b + 1], scale=1.0)
                row_ps = srow_psp.tile([1, BLK], dt.float32, tag="srow",
                                       name=f"srow{b}_{blk}")
                for uc in range(UC):
                    nc.tensor.matmul(row_ps, lhsT=v_sb[:, uc:uc + 1],
                                     rhs=tt[:, uc, :],
                                     start=(uc == 0), stop=(uc == UC - 1))
                nc.vector.tensor_copy(rowfull[0:1, blk * BLK:(blk + 1) * BLK],
                                      row_ps)
                nc.sync.dma_start(out=slab[blk:blk + 1, :],
                                  in_=rowfull[0:1, blk * BLK:(blk + 1) * BLK])

            # ---- per-batch epilogue ----
            for k in range(BLK // P):
                nc.tensor.transpose(scores_ps[:, k:NCH:BLK // P],
                                    slab[:, k * P:(k + 1) * P],
                                    ident_sb[0:NBLK, 0:NBLK])
            e_sb = small.tile([P, NCH], dt.float32, tag="e", name=f"e{b}")
            esum = small.tile([P, 1], dt.float32, tag="esum", name=f"esum{b}")
            nc.scalar.activation(out=e_sb, in_=scores_ps,
                                 func=mybir.ActivationFunctionType.Exp,
                                 accum_out=esum)
            e_bf = small.tile([P, NCH], dt.bfloat16, tag="ebf", name=f"ebf{b}")
            nc.vector.tensor_copy(e_bf, e_sb)

            s_ps = epi_psp.tile([NCH, 1], dt.float32, tag="s", name=f"s{b}")
            nc.tensor.matmul(s_ps, lhsT=ones_sb[:, 0:NCH], rhs=esum,
                             start=True, stop=True)
            rinv = small.tile([NCH, 1], dt.float32, tag="rinv", name=f"rinv{b}")
            nc.vector.reciprocal(rinv, s_ps)

            ctx_ps = ctx_psp.tile([1, d], dt.float32, tag="ctx", name=f"ctx{b}")
            for j in range(NCH):
                h, s = j // CPH, j % CPH
                nc.tensor.matmul(ctx_ps, lhsT=e_bf[:, j:j + 1],
                                 rhs=fn_tiles[h][:, s, :],
                                 start=(j == 0), stop=(j == NCH - 1))
            ctx_sb = small.tile([1, d], dt.float32, tag="ctxsb", name=f"ctxsb{b}")
            nc.vector.tensor_scalar_mul(ctx_sb, ctx_ps, rinv[0:1, :])
            nc.sync.dma_start(out=ctx_out.ap()[b:b + 1, :], in_=ctx_sb)

            et_ps = epi_psp.tile([NCH, P], dt.float32, tag="et", name=f"et{b}")
            nc.tensor.transpose(et_ps, e_sb, ident_sb)
            w_sb = small.tile([NCH, P], dt.float32, tag="wsb", name=f"wsb{b}")
            nc.scalar.mul(w_sb, et_ps, rinv)
            nc.sync.dma_start(out=attn_dst[b], in_=w_sb)

    nc.compile()
    return nc


def _get_nc():
    if "nc" not in _cache:
        _cache["nc"] = build_program()
    return _cache["nc"]


def make_in_maps(features, hidden, W1_k, W1_b, W2_k, W2_b, V_k, V_b):
    features = np.asarray(features, dtype=np.float32)
    hidden = np.asarray(hidden, dtype=np.float32)
    W1_k = np.asarray(W1_k, dtype=np.float32)
    W2_k = np.asarray(W2_k, dtype=np.float32)
    bsum = (np.asarray(W1_b, np.float32) + np.asarray(W2_b, np.float32)).reshape(U)
    vk = np.asarray(V_k, np.float32).reshape(U)
    # V_b shifts all scores of a batch equally; softmax is invariant -> drop.

    fn_all = features.astype(BF16)
    ft_all = np.ascontiguousarray(fn_all.transpose(0, 2, 1))
    w1b = W1_k.astype(BF16)

    in_maps = []
    for i in range(NCORES):
        sl = slice(i * BSH, (i + 1) * BSH)
        in_maps.append({
            "ft": ft_all[sl],
            "fn": fn_all[sl],
            "ht": np.ascontiguousarray(hidden[sl].T),
            "w1": w1b,
            "w2": W2_k,
            "bs": bsum,
            "vk": vk.astype(BF16),
        })
    return in_maps


def run_device(in_maps, trace=False):
    from concourse.bass_utils import run_bass_kernel_spmd
    nc = _get_nc()
    return run_bass_kernel_spmd(nc, in_maps, core_ids=list(range(NCORES)),
                                trace=trace)


def assemble(results):
    context = np.concatenate([r["ctx"] for r in results], axis=0)
    attn = np.concatenate([r["attn"] for r in results], axis=0)
    return context, attn.reshape(B_FULL, L, 1)


def kernel(features, hidden, W1_k, W1_b, W2_k, W2_b, V_k, V_b):
    in_maps = make_in_maps(features, hidden, W1_k, W1_b, W2_k, W2_b, V_k, V_b)
    res = run_device(in_maps, trace=False)
    return assemble(res.results)


# revision 9
# speedup vs baseline: 1.0434x; 1.0434x over previous
"""Bahdanau attention kernel for Trainium2 (8 NeuronCores, batch-parallel).

reference computation (B=64, L=4096, D=512, U=256):
    f_proj = features @ W1 + b1                    # [B, L, U]
    h_proj = hidden @ W2 + b2                      # [B, U]
    score  = tanh(f_proj + h_proj[:, None]) @ V + Vb   # [B, L, 1]
    attn   = softmax(score, axis=1)
    ctx    = sum(attn * features, axis=1)          # [B, D]
    return (ctx, attn)

Strategy: shard B across 8 cores (8 batches/core). Host pre-marshals two bf16
copies of features — natural-orientation (l on partitions, for the context
contraction over l) and transposed (d on partitions, for the f_proj
contraction over d) — since the PE contracts over the partition dim and an
on-chip full transpose is slower than streaming a second copy from HBM. Both
copies are stored pre-tiled (partition-major) so every DMA descriptor is one
16KB contiguous run. Softmax is computed without max-subtraction (scores are
bounded by sum|V| <= 16, exp stays finite in fp32), which makes the whole
thing a single fused streaming pass: raw exp(score) accumulates the context
matmul in PSUM and both outputs are normalized by 1/sum(exp) at the end. Vb
shifts every score in a batch equally, so softmax makes it a no-op and it is
dropped.

Score layout: rows [1, m] are produced with V stationary (1-column
LDWEIGHTS), staged through a [NBLK, BLK] SBUF slab (DVE copy to a partition-0
row + one 2KB SBUF->SBUF DMA per block), then 4 small PE transposes deliver
the column layout e[p, j] = exp(score[j*128+p]) whose columns are exactly the
lhsT vectors the context matmul needs. The emission order is software-
pipelined: score rows run one block behind f_proj (so the PE never waits on
the ACT tanh round-trip) and each batch's epilogue is emitted during the next
batch's block stream.
"""

import sys
from contextlib import ExitStack

if "/opt/trn_rl_repo" not in sys.path:
    sys.path.insert(0, "/opt/trn_rl_repo")

import numpy as np
import ml_dtypes

BF16 = ml_dtypes.bfloat16

B_FULL, L, D, U = 64, 4096, 512, 256
NCORES = 8
BSH = B_FULL // NCORES  # batches per core
P = 128

_cache = {}


def build_program(bsh=BSH, l=L, d=D, u=U, dma_split=2):
    import concourse.bacc as bacc
    import concourse.tile as tile
    from concourse import mybir
    from concourse.masks import make_identity

    dt = mybir.dt
    DC = d // P            # d-chunks (4)
    UC = u // P            # u-chunks (2)
    NCH = l // P           # l-chunks per batch (32)
    BLK = 512              # m-block for the f_proj matmuls
    NBLK = l // BLK        # blocks per batch (8)
    CPB = BLK // P         # l-chunks per block (4)
    MH = l // dma_split    # m per DMA (2048)
    CPH = MH // P          # l-chunks per DMA half (16)
    BPH = MH // BLK        # blocks per DMA half (4)

    nc = bacc.Bacc("TRN2", target_bir_lowering=False, debug=False,
                   num_devices=NCORES)

    # pre-tiled (partition-major) feature copies — every DMA descriptor is a
    # contiguous per-partition run
    ft = nc.dram_tensor("ft", [bsh, P, DC, l], dt.bfloat16, kind="ExternalInput")
    fn = nc.dram_tensor("fn", [bsh, P, NCH, d], dt.bfloat16, kind="ExternalInput")
    ht = nc.dram_tensor("ht", [d, bsh], dt.float32, kind="ExternalInput")
    w1 = nc.dram_tensor("w1", [d, u], dt.bfloat16, kind="ExternalInput")
    w2 = nc.dram_tensor("w2", [d, u], dt.float32, kind="ExternalInput")
    bs = nc.dram_tensor("bs", [u], dt.float32, kind="ExternalInput")
    vk = nc.dram_tensor("vk", [u], dt.bfloat16, kind="ExternalInput")
    ctx_out = nc.dram_tensor("ctx", [bsh, d], dt.float32, kind="ExternalOutput")
    attn_out = nc.dram_tensor("attn", [bsh, l], dt.float32, kind="ExternalOutput")

    attn_dst = attn_out.ap().rearrange("b (j p) -> b j p", p=P)

    with tile.TileContext(nc) as tc, ExitStack() as es:
        const = es.enter_context(tc.tile_pool(name="const", bufs=1))
        ftp = es.enter_context(tc.tile_pool(name="ftp", bufs=3))
        fnp = es.enter_context(tc.tile_pool(name="fnp", bufs=dma_split * 2 + 1))
        ttp = es.enter_context(tc.tile_pool(name="ttp", bufs=3))
        small = es.enter_context(tc.tile_pool(name="small", bufs=4))
        rowp = es.enter_context(tc.tile_pool(name="rowp", bufs=1))
        slabp = es.enter_context(tc.tile_pool(name="slabp", bufs=2))
        fproj_ps = es.enter_context(tc.tile_pool(name="fproj_ps", bufs=2, space="PSUM"))
        scores_psp = es.enter_context(tc.tile_pool(name="scores_ps", bufs=1, space="PSUM"))
        srow_psp = es.enter_context(tc.tile_pool(name="srow_ps", bufs=2, space="PSUM"))
        ctx_psp = es.enter_context(tc.tile_pool(name="ctx_ps", bufs=1, space="PSUM"))
        epi_psp = es.enter_context(tc.tile_pool(name="epi_ps", bufs=1, space="PSUM"))

        # ---- constants ----
        w1_sb = const.tile([P, DC, u], dt.bfloat16, name="w1_sb")
        nc.sync.dma_start(out=w1_sb, in_=w1.ap().rearrange("(dc p) u -> p dc u", p=P))
        w2_sb = const.tile([P, DC, u], dt.float32, name="w2_sb")
        nc.sync.dma_start(out=w2_sb, in_=w2.ap().rearrange("(dc p) u -> p dc u", p=P))
        ht_sb = const.tile([P, DC, bsh], dt.float32, name="ht_sb")
        nc.sync.dma_start(out=ht_sb, in_=ht.ap().rearrange("(dc p) b -> p dc b", p=P))
        v_sb = const.tile([P, UC], dt.bfloat16, name="v_sb")
        nc.sync.dma_start(out=v_sb, in_=vk.ap().rearrange("(uc p) -> p uc", p=P))
        bs_sb = const.tile([P, UC], dt.float32, name="bs_sb")
        nc.sync.dma_start(out=bs_sb, in_=bs.ap().rearrange("(uc p) -> p uc", p=P))
        ones_sb = const.tile([P, NCH], dt.float32, name="ones_sb")
        nc.vector.memset(ones_sb, 1.0)
        ident_sb = const.tile([P, P], dt.float32, name="ident_sb")
        make_identity(nc, ident_sb)
        qt_sb = const.tile([P, UC, bsh], dt.float32, name="qt_sb")

        # ---- h_proj (tiny): qT[u, b] = (W2.T @ hT)[u, b] + b1[u] + b2[u] ----
        for uc in range(UC):
            q_ps = epi_psp.tile([P, bsh], dt.float32, tag="et", name=f"q_ps{uc}")
            for dc in range(DC):
                nc.tensor.matmul(q_ps, lhsT=w2_sb[:, dc, uc * P:(uc + 1) * P],
                                 rhs=ht_sb[:, dc, :],
                                 start=(dc == 0), stop=(dc == DC - 1))
            nc.vector.tensor_scalar_add(qt_sb[:, uc, :], q_ps,
                                        bs_sb[:, uc:uc + 1])

        # per-batch state carried across the pipelined emission
        state = {}

        def emit_block(b, blk):
            st = state[b]
            h, moff = blk // BPH, (blk % BPH) * BLK
            ftt = st["ft"][h]
            fp_ps = []
            for uc in range(UC):
                fp = fproj_ps.tile([P, BLK], dt.float32, tag="fproj",
                                   name=f"fp{b}_{blk}_{uc}")
                for dc in range(DC):
                    nc.tensor.matmul(fp, lhsT=w1_sb[:, dc, uc * P:(uc + 1) * P],
                                     rhs=ftt[:, dc, moff:moff + BLK],
                                     start=(dc == 0), stop=(dc == DC - 1))
                fp_ps.append(fp)
            tt = ttp.tile([P, UC, BLK], dt.bfloat16, tag="tt", name=f"tt{b}_{blk}")
            for uc in range(UC):
                nc.scalar.activation(out=tt[:, uc, :], in_=fp_ps[uc],
                                     func=mybir.ActivationFunctionType.Tanh,
                                     bias=qt_sb[:, uc, b:b + 1], scale=1.0)
            st["tt"][blk] = tt

        def emit_srow(b, blk):
            st = state[b]
            tt = st["tt"][blk]
            row_ps = srow_psp.tile([1, BLK], dt.float32, tag="srow",
                                   name=f"srow{b}_{blk}")
            for uc in range(UC):
                nc.tensor.matmul(row_ps, lhsT=v_sb[:, uc:uc + 1],
                                 rhs=tt[:, uc, :],
                                 start=(uc == 0), stop=(uc == UC - 1))
            rowfull = st["rowfull"]
            nc.vector.tensor_copy(rowfull[0:1, blk * BLK:(blk + 1) * BLK], row_ps)
            nc.sync.dma_start(out=st["slab"][blk:blk + 1, :],
                              in_=rowfull[0:1, blk * BLK:(blk + 1) * BLK])
            st["tt"][blk] = None

        def emit_epilogue(b):
            st = state[b]
            scores_ps = scores_psp.tile([P, NCH], dt.float32, tag="scores",
                                        name=f"scores{b}")
            for k in range(CPB):
                nc.tensor.transpose(scores_ps[:, k:NCH:CPB],
                                    st["slab"][:, k * P:(k + 1) * P],
                                    ident_sb[0:NBLK, 0:NBLK])
            e_sb = small.tile([P, NCH], dt.float32, tag="e", name=f"e{b}")
            esum = small.tile([P, 1], dt.float32, tag="esum", name=f"esum{b}")
            nc.scalar.activation(out=e_sb, in_=scores_ps,
                                 func=mybir.ActivationFunctionType.Exp,
                                 accum_out=esum)
            e_bf = small.tile([P, NCH], dt.bfloat16, tag="ebf", name=f"ebf{b}")
            nc.vector.tensor_copy(e_bf, e_sb)

            s_ps = epi_psp.tile([NCH, 1], dt.float32, tag="s", name=f"s{b}")
            nc.tensor.matmul(s_ps, lhsT=ones_sb[:, 0:NCH], rhs=esum,
                             start=True, stop=True)
            rinv = small.tile([NCH, 1], dt.float32, tag="rinv", name=f"rinv{b}")
            nc.vector.reciprocal(rinv, s_ps)

            ctx_ps = ctx_psp.tile([1, d], dt.float32, tag="ctx", name=f"ctx{b}")
            for j in range(NCH):
                h, s = j // CPH, j % CPH
                nc.tensor.matmul(ctx_ps, lhsT=e_bf[:, j:j + 1],
                                 rhs=st["fn"][h][:, s, :],
                                 start=(j == 0), stop=(j == NCH - 1))
            ctx_sb = small.tile([1, d], dt.float32, tag="ctxsb", name=f"ctxsb{b}")
            nc.vector.tensor_scalar_mul(ctx_sb, ctx_ps, rinv[0:1, :])
            nc.sync.dma_start(out=ctx_out.ap()[b:b + 1, :], in_=ctx_sb)

            et_ps = epi_psp.tile([NCH, P], dt.float32, tag="et", name=f"et{b}")
            nc.tensor.transpose(et_ps, e_sb, ident_sb)
            w_sb = small.tile([NCH, P], dt.float32, tag="wsb", name=f"wsb{b}")
            nc.scalar.mul(w_sb, et_ps, rinv)
            nc.sync.dma_start(out=attn_dst[b], in_=w_sb)
            state[b] = None

        # ---- software-pipelined emission over this core's batches ----
        for b in range(bsh):
            st = state[b] = {"ft": [], "fn": [], "tt": [None] * NBLK}
            for h in range(dma_split):
                ftt = ftp.tile([P, DC, MH], dt.bfloat16, tag="ft", name=f"ft{b}_{h}")
                nc.sync.dma_start(out=ftt, in_=ft.ap()[b, :, :, h * MH:(h + 1) * MH])
                st["ft"].append(ftt)
                fnt = fnp.tile([P, CPH, d], dt.bfloat16, tag="fn", name=f"fn{b}_{h}")
                nc.gpsimd.dma_start(out=fnt,
                                    in_=fn.ap()[b, :, h * CPH:(h + 1) * CPH, :])
                st["fn"].append(fnt)
            st["slab"] = slabp.tile([NBLK, BLK], dt.float32, tag="slab",
                                    name=f"slab{b}")
            st["rowfull"] = rowp.tile([1, l], dt.float32, tag="rowfull",
                                      name=f"rowfull{b}")

            for blk in range(NBLK):
                emit_block(b, blk)
                if blk > 0:
                    emit_srow(b, blk - 1)
                if b > 0 and blk == 1:
                    emit_epilogue(b - 1)
            emit_srow(b, NBLK - 1)
        emit_epilogue(bsh - 1)

    nc.compile()
    return nc


def _get_nc():
    if "nc" not in _cache:
        _cache["nc"] = build_program()
    return _cache["nc"]


def make_in_maps(features, hidden, W1_k, W1_b, W2_k, W2_b, V_k, V_b):
    features = np.asarray(features, dtype=np.float32)
    hidden = np.asarray(hidden, dtype=np.float32)
    W1_k = np.asarray(W1_k, dtype=np.float32)
    W2_k = np.asarray(W2_k, dtype=np.float32)
    bsum = (np.asarray(W1_b, np.float32) + np.asarray(W2_b, np.float32)).reshape(U)
    vk = np.asarray(V_k, np.float32).reshape(U)
    # V_b shifts all scores of a batch equally; softmax is invariant -> drop.

    fb = features.astype(BF16)
    # fn pre-tiled: [B, P, NCH, D]; partition line = NCH*D*2 contiguous bytes
    fn_all = np.ascontiguousarray(
        fb.reshape(B_FULL, L // P, P, D).transpose(0, 2, 1, 3))
    # ft pre-tiled: [B, P, DC, L]; partition line = DC*L*2 contiguous bytes
    ftr = np.ascontiguousarray(fb.transpose(0, 2, 1))  # [B, D, L]
    ft_all = np.ascontiguousarray(
        ftr.reshape(B_FULL, D // P, P, L).transpose(0, 2, 1, 3))
    w1b = W1_k.astype(BF16)

    in_maps = []
    for i in range(NCORES):
        sl = slice(i * BSH, (i + 1) * BSH)
        in_maps.append({
            "ft": ft_all[sl],
            "fn": fn_all[sl],
            "ht": np.ascontiguousarray(hidden[sl].T),
            "w1": w1b,
            "w2": W2_k,
            "bs": bsum,
            "vk": vk.astype(BF16),
        })
    return in_maps


def run_device(in_maps, trace=False):
    from concourse.bass_utils import run_bass_kernel_spmd
    nc = _get_nc()
    return run_bass_kernel_spmd(nc, in_maps, core_ids=list(range(NCORES)),
                                trace=trace)


def assemble(results):
    context = np.concatenate([r["ctx"] for r in results], axis=0)
    attn = np.concatenate([r["attn"] for r in results], axis=0)
    return context, attn.reshape(B_FULL, L, 1)


def kernel(features, hidden, W1_k, W1_b, W2_k, W2_b, V_k, V_b):
    in_maps = make_in_maps(features, hidden, W1_k, W1_b, W2_k, W2_b, V_k, V_b)
    res = run_device(in_maps, trace=False)
    return assemble(res.results)


# revision 11
# speedup vs baseline: 1.1121x; 1.0658x over previous
"""Bahdanau attention kernel for Trainium2 (8 NeuronCores, batch-parallel).

reference computation (B=64, L=4096, D=512, U=256):
    f_proj = features @ W1 + b1                    # [B, L, U]
    h_proj = hidden @ W2 + b2                      # [B, U]
    score  = tanh(f_proj + h_proj[:, None]) @ V + Vb   # [B, L, 1]
    attn   = softmax(score, axis=1)
    ctx    = sum(attn * features, axis=1)          # [B, D]
    return (ctx, attn)

Strategy: shard B across 8 cores (8 batches/core). Host pre-marshals two bf16
copies of features — natural-orientation (l on partitions, for the context
contraction over l) and transposed (d on partitions, for the f_proj
contraction over d) — since the PE contracts over the partition dim and an
on-chip full transpose is slower than streaming a second copy from HBM. Both
copies are stored pre-tiled (partition-major) so every DMA descriptor is one
16KB contiguous run. Softmax is computed without max-subtraction (scores are
bounded by sum|V| <= 16, exp stays finite in fp32), which makes the whole
thing a single fused streaming pass: raw exp(score) accumulates the context
matmul in PSUM and both outputs are normalized by 1/sum(exp) at the end. Vb
shifts every score in a batch equally, so softmax makes it a no-op and it is
dropped.

Score layout: rows [1, m] are produced with V stationary (1-column
LDWEIGHTS), staged through a [NBLK, BLK] SBUF slab (DVE copy to a partition-0
row + one 2KB SBUF->SBUF DMA per block), then 4 small PE transposes deliver
the column layout e[p, j] = exp(score[j*128+p]) whose columns are exactly the
lhsT vectors the context matmul needs. The emission order is software-
pipelined: score rows run one block behind f_proj (so the PE never waits on
the ACT tanh round-trip) and each batch's epilogue is emitted during the next
batch's block stream.
"""

import sys
from contextlib import ExitStack

if "/opt/trn_rl_repo" not in sys.path:
    sys.path.insert(0, "/opt/trn_rl_repo")

import numpy as np
import ml_dtypes

BF16 = ml_dtypes.bfloat16

B_FULL, L, D, U = 64, 4096, 512, 256
NCORES = 8
BSH = B_FULL // NCORES  # batches per core
P = 128

_cache = {}


def build_program(bsh=BSH, l=L, d=D, u=U, dma_split=2):
    import concourse.bacc as bacc
    import concourse.tile as tile
    from concourse import mybir
    from concourse.masks import make_identity

    dt = mybir.dt
    DC = d // P            # d-chunks (4)
    UC = u // P            # u-chunks (2)
    NCH = l // P           # l-chunks per batch (32)
    BLK = 512              # m-block for the f_proj matmuls
    NBLK = l // BLK        # blocks per batch (8)
    CPB = BLK // P         # l-chunks per block (4)
    MH = l // dma_split    # m per DMA (2048)
    CPH = MH // P          # l-chunks per DMA half (16)
    BPH = MH // BLK        # blocks per DMA half (4)

    nc = bacc.Bacc("TRN2", target_bir_lowering=False, debug=False,
                   num_devices=NCORES)

    # pre-tiled (partition-major) feature copies — every DMA descriptor is a
    # contiguous per-partition run
    ft = nc.dram_tensor("ft", [bsh, P, DC, l], dt.bfloat16, kind="ExternalInput")
    fn = nc.dram_tensor("fn", [bsh, P, NCH, d], dt.bfloat16, kind="ExternalInput")
    ht = nc.dram_tensor("ht", [d, bsh], dt.float32, kind="ExternalInput")
    w1 = nc.dram_tensor("w1", [d, u], dt.bfloat16, kind="ExternalInput")
    w2 = nc.dram_tensor("w2", [d, u], dt.float32, kind="ExternalInput")
    bs = nc.dram_tensor("bs", [u], dt.float32, kind="ExternalInput")
    vk = nc.dram_tensor("vk", [u], dt.bfloat16, kind="ExternalInput")
    ctx_out = nc.dram_tensor("ctx", [bsh, d], dt.float32, kind="ExternalOutput")
    attn_out = nc.dram_tensor("attn", [bsh, l], dt.float32, kind="ExternalOutput")

    attn_dst = attn_out.ap().rearrange("b (j p) -> b j p", p=P)

    with tile.TileContext(nc) as tc, ExitStack() as es:
        const = es.enter_context(tc.tile_pool(name="const", bufs=1))
        ftp = es.enter_context(tc.tile_pool(name="ftp", bufs=3))
        fnp = es.enter_context(tc.tile_pool(name="fnp", bufs=dma_split * 2 + 1))
        ttp = es.enter_context(tc.tile_pool(name="ttp", bufs=3))
        small = es.enter_context(tc.tile_pool(name="small", bufs=4))
        rowp = es.enter_context(tc.tile_pool(name="rowp", bufs=1))
        slabp = es.enter_context(tc.tile_pool(name="slabp", bufs=2))
        fproj_ps = es.enter_context(tc.tile_pool(name="fproj_ps", bufs=3, space="PSUM"))
        scores_psp = es.enter_context(tc.tile_pool(name="scores_ps", bufs=1, space="PSUM"))
        srow_psp = es.enter_context(tc.tile_pool(name="srow_ps", bufs=1, space="PSUM"))
        ctx_psp = es.enter_context(tc.tile_pool(name="ctx_ps", bufs=1, space="PSUM"))
        epi_psp = es.enter_context(tc.tile_pool(name="epi_ps", bufs=1, space="PSUM"))

        # ---- constants ----
        w1_sb = const.tile([P, DC, u], dt.bfloat16, name="w1_sb")
        nc.sync.dma_start(out=w1_sb, in_=w1.ap().rearrange("(dc p) u -> p dc u", p=P))
        w2_sb = const.tile([P, DC, u], dt.float32, name="w2_sb")
        nc.sync.dma_start(out=w2_sb, in_=w2.ap().rearrange("(dc p) u -> p dc u", p=P))
        ht_sb = const.tile([P, DC, bsh], dt.float32, name="ht_sb")
        nc.sync.dma_start(out=ht_sb, in_=ht.ap().rearrange("(dc p) b -> p dc b", p=P))
        v_sb = const.tile([P, UC], dt.bfloat16, name="v_sb")
        nc.sync.dma_start(out=v_sb, in_=vk.ap().rearrange("(uc p) -> p uc", p=P))
        bs_sb = const.tile([P, UC], dt.float32, name="bs_sb")
        nc.sync.dma_start(out=bs_sb, in_=bs.ap().rearrange("(uc p) -> p uc", p=P))
        ones_sb = const.tile([P, NCH], dt.float32, name="ones_sb")
        nc.vector.memset(ones_sb, 1.0)
        ident_sb = const.tile([P, P], dt.float32, name="ident_sb")
        make_identity(nc, ident_sb)
        qt_sb = const.tile([P, UC, bsh], dt.float32, name="qt_sb")

        # ---- h_proj (tiny): qT[u, b] = (W2.T @ hT)[u, b] + b1[u] + b2[u] ----
        for uc in range(UC):
            q_ps = epi_psp.tile([P, bsh], dt.float32, tag="et", name=f"q_ps{uc}")
            for dc in range(DC):
                nc.tensor.matmul(q_ps, lhsT=w2_sb[:, dc, uc * P:(uc + 1) * P],
                                 rhs=ht_sb[:, dc, :],
                                 start=(dc == 0), stop=(dc == DC - 1))
            nc.vector.tensor_scalar_add(qt_sb[:, uc, :], q_ps,
                                        bs_sb[:, uc:uc + 1])

        # per-batch state carried across the pipelined emission
        state = {}

        def emit_block(b, blk):
            st = state[b]
            h, moff = blk // BPH, (blk % BPH) * BLK
            ftt = st["ft"][h]
            fp_ps = []
            for uc in range(UC):
                fp = fproj_ps.tile([P, BLK], dt.float32, tag="fproj",
                                   name=f"fp{b}_{blk}_{uc}")
                for dc in range(DC):
                    nc.tensor.matmul(fp, lhsT=w1_sb[:, dc, uc * P:(uc + 1) * P],
                                     rhs=ftt[:, dc, moff:moff + BLK],
                                     start=(dc == 0), stop=(dc == DC - 1))
                fp_ps.append(fp)
            tt = ttp.tile([P, UC, BLK], dt.bfloat16, tag="tt", name=f"tt{b}_{blk}")
            for uc in range(UC):
                nc.scalar.activation(out=tt[:, uc, :], in_=fp_ps[uc],
                                     func=mybir.ActivationFunctionType.Tanh,
                                     bias=qt_sb[:, uc, b:b + 1], scale=1.0)
            st["tt"][blk] = tt

        def emit_srow(b, blk):
            st = state[b]
            tt = st["tt"][blk]
            row_ps = srow_psp.tile([1, BLK], dt.float32, tag="srow",
                                   name=f"srow{b}_{blk}")
            for uc in range(UC):
                nc.tensor.matmul(row_ps, lhsT=v_sb[:, uc:uc + 1],
                                 rhs=tt[:, uc, :],
                                 start=(uc == 0), stop=(uc == UC - 1))
            rowfull = st["rowfull"]
            nc.vector.tensor_copy(rowfull[0:1, blk * BLK:(blk + 1) * BLK], row_ps)
            nc.sync.dma_start(out=st["slab"][blk:blk + 1, :],
                              in_=rowfull[0:1, blk * BLK:(blk + 1) * BLK])
            st["tt"][blk] = None

        def emit_epilogue_a(b):
            st = state[b]
            scores_ps = scores_psp.tile([P, NCH], dt.float32, tag="scores",
                                        name=f"scores{b}")
            for k in range(CPB):
                nc.tensor.transpose(scores_ps[:, k:NCH:CPB],
                                    st["slab"][:, k * P:(k + 1) * P],
                                    ident_sb[0:NBLK, 0:NBLK])
            e_sb = small.tile([P, NCH], dt.float32, tag="e", name=f"e{b}")
            esum = small.tile([P, 1], dt.float32, tag="esum", name=f"esum{b}")
            nc.scalar.activation(out=e_sb, in_=scores_ps,
                                 func=mybir.ActivationFunctionType.Exp,
                                 accum_out=esum)
            e_bf = small.tile([P, NCH], dt.bfloat16, tag="ebf", name=f"ebf{b}")
            nc.vector.tensor_copy(e_bf, e_sb)

            s_ps = epi_psp.tile([NCH, 1], dt.float32, tag="s", name=f"s{b}")
            nc.tensor.matmul(s_ps, lhsT=ones_sb[:, 0:NCH], rhs=esum,
                             start=True, stop=True)
            rinv = small.tile([NCH, 1], dt.float32, tag="rinv", name=f"rinv{b}")
            nc.vector.reciprocal(rinv, s_ps)
            st["e_sb"], st["e_bf"], st["rinv"] = e_sb, e_bf, rinv

        def emit_epilogue(b):
            st = state[b]
            e_sb, e_bf, rinv = st["e_sb"], st["e_bf"], st["rinv"]
            ctx_ps = ctx_psp.tile([1, d], dt.float32, tag="ctx", name=f"ctx{b}")
            for j in range(NCH):
                h, s = j // CPH, j % CPH
                nc.tensor.matmul(ctx_ps, lhsT=e_bf[:, j:j + 1],
                                 rhs=st["fn"][h][:, s, :],
                                 start=(j == 0), stop=(j == NCH - 1))
            ctx_sb = small.tile([1, d], dt.float32, tag="ctxsb", name=f"ctxsb{b}")
            nc.vector.tensor_scalar_mul(ctx_sb, ctx_ps, rinv[0:1, :])
            nc.sync.dma_start(out=ctx_out.ap()[b:b + 1, :], in_=ctx_sb)

            et_ps = epi_psp.tile([NCH, P], dt.float32, tag="et", name=f"et{b}")
            nc.tensor.transpose(et_ps, e_sb, ident_sb)
            w_sb = small.tile([NCH, P], dt.float32, tag="wsb", name=f"wsb{b}")
            nc.scalar.mul(w_sb, et_ps, rinv)
            nc.sync.dma_start(out=attn_dst[b], in_=w_sb)
            state[b] = None

        # ---- software-pipelined emission over this core's batches ----
        for b in range(bsh):
            st = state[b] = {"ft": [], "fn": [], "tt": [None] * NBLK}
            for h in range(dma_split):
                ftt = ftp.tile([P, DC, MH], dt.bfloat16, tag="ft", name=f"ft{b}_{h}")
                nc.sync.dma_start(out=ftt, in_=ft.ap()[b, :, :, h * MH:(h + 1) * MH])
                st["ft"].append(ftt)
            st["slab"] = slabp.tile([NBLK, BLK], dt.float32, tag="slab",
                                    name=f"slab{b}")
            st["rowfull"] = rowp.tile([1, l], dt.float32, tag="rowfull",
                                      name=f"rowfull{b}")

            for blk in range(NBLK):
                emit_block(b, blk)
                while (len(st["fn"]) < dma_split
                       and blk >= min(2 + len(st["fn"]), NBLK - 1)):
                    h = len(st["fn"])
                    fnt = fnp.tile([P, CPH, d], dt.bfloat16, tag="fn",
                                   name=f"fn{b}_{h}")
                    nc.gpsimd.dma_start(out=fnt,
                                        in_=fn.ap()[b, :, h * CPH:(h + 1) * CPH, :])
                    st["fn"].append(fnt)
                if blk > 0:
                    emit_srow(b, blk - 1)
                if b > 0 and blk == 1:
                    emit_epilogue_a(b - 1)
                if b > 0 and blk == 2:
                    emit_epilogue(b - 1)
            emit_srow(b, NBLK - 1)
        emit_epilogue_a(bsh - 1)
        emit_epilogue(bsh - 1)

    nc.compile()
    return nc


def _get_nc():
    if "nc" not in _cache:
        _cache["nc"] = build_program()
    return _cache["nc"]


def make_in_maps(features, hidden, W1_k, W1_b, W2_k, W2_b, V_k, V_b):
    features = np.asarray(features, dtype=np.float32)
    hidden = np.asarray(hidden, dtype=np.float32)
    W1_k = np.asarray(W1_k, dtype=np.float32)
    W2_k = np.asarray(W2_k, dtype=np.float32)
    bsum = (np.asarray(W1_b, np.float32) + np.asarray(W2_b, np.float32)).reshape(U)
    vk = np.asarray(V_k, np.float32).reshape(U)
    # V_b shifts all scores of a batch equally; softmax is invariant -> drop.

    fb = features.astype(BF16)
    # fn pre-tiled: [B, P, NCH, D]; partition line = NCH*D*2 contiguous bytes
    fn_all = np.ascontiguousarray(
        fb.reshape(B_FULL, L // P, P, D).transpose(0, 2, 1, 3))
    # ft pre-tiled: [B, P, DC, L]; partition line = DC*L*2 contiguous bytes
    ftr = np.ascontiguousarray(fb.transpose(0, 2, 1))  # [B, D, L]
    ft_all = np.ascontiguousarray(
        ftr.reshape(B_FULL, D // P, P, L).transpose(0, 2, 1, 3))
    w1b = W1_k.astype(BF16)

    in_maps = []
    for i in range(NCORES):
        sl = slice(i * BSH, (i + 1) * BSH)
        in_maps.append({
            "ft": ft_all[sl],
            "fn": fn_all[sl],
            "ht": np.ascontiguousarray(hidden[sl].T),
            "w1": w1b,
            "w2": W2_k,
            "bs": bsum,
            "vk": vk.astype(BF16),
        })
    return in_maps


def run_device(in_maps, trace=False):
    from concourse.bass_utils import run_bass_kernel_spmd
    nc = _get_nc()
    return run_bass_kernel_spmd(nc, in_maps, core_ids=list(range(NCORES)),
                                trace=trace)


def assemble(results):
    context = np.concatenate([r["ctx"] for r in results], axis=0)
    attn = np.concatenate([r["attn"] for r in results], axis=0)
    return context, attn.reshape(B_FULL, L, 1)


def kernel(features, hidden, W1_k, W1_b, W2_k, W2_b, V_k, V_b):
    in_maps = make_in_maps(features, hidden, W1_k, W1_b, W2_k, W2_b, V_k, V_b)
    res = run_device(in_maps, trace=False)
    return assemble(res.results)
